# revision 1
# baseline (speedup 1.0000x reference)
"""GATv2 classifier kernel for Trainium2, 8-core SPMD.

Strategy:
  - Nodes are dealt round-robin by descending in-degree across 8 cores.
    Edges partitioned by destination -> segment softmax and aggregation
    stay core-local. No collectives: every core redundantly computes the
    full xl = x@Wl+bl table (cheap matmul).
  - dma_gather indices are int16 (max 32767 < N), so xl lives in TWO
    DRAM tables split at node LO = 63*512, each with a zero row used by
    padding slots. To avoid gathering every slot from both tables, the
    work is split into TWO PASSES: pass L processes only lo-source
    edges (nodes re-bucketed by lo-degree, gathers from table_lo only),
    pass H only hi-source edges. Pass L writes partial (sum p*z, sum p)
    per node to DRAM; pass H merges them via one cheap 6272-row gather.
  - Per bucket (128 dst nodes x K padded slots): z = xl[src] + xr[dst]
    is built on the TensorEngine in PSUM (identity matmul accumulating
    the gathered rows + xrT matmul with a replicated identity).
  - e = sum_f att * leaky_relu(z)  (ACT Prelu, exact on HW);
    p = exp(e) * mask  (no max subtraction: |e| <~ 6 here, exp safe).
  - h = (sum p*z) / (sum p) - xr  removes the xr contribution
    algebraically, so gathered xl never needs to be kept separately.
  - logit = rden * sum_f(agg*wo) - xrwo, with xrwo = (x@Wr+br)@Wo
    precomputed on host; out = sigmoid(logit + bo'), bo' = bo + bias@Wo.
"""

import math
import os
import sys

import numpy as np

if os.path.isdir("/opt/trn_rl_repo") and "/opt/trn_rl_repo" not in sys.path:
    sys.path.insert(0, "/opt/trn_rl_repo")

P = 128
NEG_SLOPE = 0.2
CHUNK = 512          # nodes per phase-1 table-build chunk
PARTW = 192          # fp32 words per partial row (128 agg + 1 den + pad), 768B


# --------------------------------------------------------------------------
# Host-side planning
# --------------------------------------------------------------------------

def _wrap_idx(grid):
    """[K,128] slot grid -> dma_gather wrapped idx layout [128, K*128/16]."""
    flat = grid.reshape(-1).astype(np.int16)
    return np.tile(flat.reshape(-1, 16).T, (8, 1))


def _plan(x, edge_index, Wl, bl, Wr, br, att, bias, Wo, bo, n_cores=8):
    N, F = x.shape
    assert F == P
    C = n_cores

    src = np.concatenate([np.asarray(edge_index[0], dtype=np.int64),
                          np.arange(N, dtype=np.int64)])
    dst = np.concatenate([np.asarray(edge_index[1], dtype=np.int64),
                          np.arange(N, dtype=np.int64)])
    deg = np.bincount(dst, minlength=N)

    # CSR by destination, lo-source edges first within each segment
    n_chunks = (N + CHUNK - 1) // CHUNK
    N_pad = n_chunks * CHUNK
    L_lo = min(n_chunks, 63)
    LO = L_lo * CHUNK
    e_order = np.lexsort((src >= LO, dst))     # by dst, lo srcs first
    src_sorted = src[e_order]
    starts = np.concatenate([[0], np.cumsum(deg)]).astype(np.int64)
    lo_cnt = np.bincount(dst, weights=(src < LO).astype(np.float64),
                         minlength=N).astype(np.int64)
    hi_cnt = deg - lo_cnt

    # deal nodes round-robin by descending total degree
    order = np.argsort(-deg, kind="stable")
    npc = (N + C - 1) // C
    NB = (npc + P - 1) // P
    npc_pad = NB * P
    order_pad = np.full(C * npc_pad, -1, dtype=np.int64)
    order_pad[:N] = order
    core_nodes = np.stack([order_pad[c::C] for c in range(C)])  # [C, npc_pad]

    # per-pass orderings within each core (positions into core_nodes[c])
    def pass_order(cnt):
        orders = np.zeros((C, npc_pad), dtype=np.int64)
        for c in range(C):
            nodes = core_nodes[c]
            key = np.where(nodes >= 0, cnt[np.maximum(nodes, 0)], -1)
            orders[c] = np.argsort(-key, kind="stable")
        return orders

    ordL = pass_order(lo_cnt)   # positions, lo-degree descending
    ordH = pass_order(hi_cnt)

    def k_sched(cnt, orders):
        Ks = []
        for b in range(NB):
            m = 1
            for c in range(C):
                nodes = core_nodes[c][orders[c][b * P:(b + 1) * P]]
                ok = nodes >= 0
                if ok.any():
                    m = max(m, int(cnt[nodes[ok]].max()))
            Ks.append(max(4, ((m + 3) // 4) * 4))
        return Ks

    KsL = k_sched(lo_cnt, ordL)
    KsH = k_sched(hi_cnt, ordH)

    def offs(Ks):
        so, ko, s, k = [], [], 0, 0
        for K in Ks:
            so.append(s)
            ko.append(k)
            s += (K * P) // 16
            k += K
        return so, ko, s, k

    soL, koL, StotL16, KtotL = offs(KsL)
    soH, koH, StotH16, KtotH = offs(KsH)

    xT16 = np.asarray(x, dtype=np.float16).T            # [128, N]

    idxL = np.zeros((C, P, StotL16), dtype=np.int16)
    idxH = np.zeros((C, P, StotH16), dtype=np.int16)
    maskL = np.zeros((C, P, KtotL), dtype=np.float16)
    maskH = np.zeros((C, P, KtotH), dtype=np.float16)
    xT_L = np.zeros((C, P, npc_pad), dtype=np.float16)
    xT_H = np.zeros((C, P, npc_pad), dtype=np.float16)
    merge_idx = np.zeros((C, P, npc_pad // 16), dtype=np.int16)
    xrwo_col = np.zeros((C, P, NB), dtype=np.float32)

    # host-precomputed (x@Wr+br)@Wo  [N]
    xrwo = ((np.asarray(x, dtype=np.float64) @ np.asarray(Wr, dtype=np.float64)
             + np.asarray(br, dtype=np.float64))
            @ np.asarray(Wo, dtype=np.float64)[:, 0])

    for c in range(C):
        nodes = core_nodes[c]
        posL_of = np.empty(npc_pad, dtype=np.int64)
        posL_of[ordL[c]] = np.arange(npc_pad)

        for (idx_a, mask_a, xt_a, Ks, so_a, ko_a, orders, cnt, base, is_lo) in (
            (idxL, maskL, xT_L, KsL, soL, koL, ordL, lo_cnt, 0, True),
            (idxH, maskH, xT_H, KsH, soH, koH, ordH, hi_cnt, LO, False),
        ):
            o = orders[c]
            nds = nodes[o]                      # node id per position
            ok = nds >= 0
            xt_a[c][:, ok] = xT16[:, nds[ok]]
            cnts = np.where(ok, cnt[np.maximum(nds, 0)], 0)
            # offset of this pass's edges within the node's CSR segment:
            # lo edges come first (lexsort), hi edges after lo_cnt
            seg0 = starts[np.maximum(nds, 0)] + (0 if is_lo else
                                                 lo_cnt[np.maximum(nds, 0)])
            for b in range(NB):
                K = Ks[b]
                sl = slice(b * P, (b + 1) * P)
                db = cnts[sl]
                kk = np.arange(K)[:, None]                     # [K, 128]
                valid = kk < db[None, :]
                pos = seg0[sl][None, :] + kk
                srcg = np.where(valid,
                                src_sorted[np.minimum(pos, len(src_sorted) - 1)], 0)
                rel = np.where(valid, srcg - base + 1, 0)
                m = valid.astype(np.float16)
                if is_lo:
                    m[0, (nds[sl] < 0)] = 1.0    # dummy keeps den >= 1
                idx_a[c][:, so_a[b]:so_a[b] + (K * P) // 16] = _wrap_idx(rel)
                mask_a[c][:, ko_a[b]:ko_a[b] + K] = m.T
            if not is_lo:
                # merge gather: H-position -> L-position row of partials
                merge_idx[c] = _wrap_idx(posL_of[o].reshape(-1, P)
                                         .reshape(npc_pad // P, P))
                xrw = np.where(ok, xrwo[np.maximum(nds, 0)], 0.0)
                xrwo_col[c] = xrw.reshape(NB, P).T.astype(np.float32)

    xT_full = np.zeros((P, N_pad), dtype=np.float16)
    xT_full[:, :N] = xT16

    wl = np.asarray(Wl, dtype=np.float16)
    wr = np.asarray(Wr, dtype=np.float16)
    bl_row = np.asarray(bl, dtype=np.float16).reshape(1, P)
    br_row = np.asarray(br, dtype=np.float16).reshape(1, P)
    att16 = np.asarray(att, dtype=np.float16)
    att_rep = np.tile(att16[None, None, :], (P, 4, 1))           # [128,4,128]
    i_rep = np.tile(np.eye(P, dtype=np.float16)[:, None, :], (1, 4, 1))
    wo_rep = np.tile(np.asarray(Wo, dtype=np.float16)[:, 0][None, :], (P, 1))
    bo_eff = float(np.asarray(bo).reshape(-1)[0] +
                   np.asarray(bias, dtype=np.float64)
                   @ np.asarray(Wo, dtype=np.float64)[:, 0])

    cfg = dict(N=N, C=C, NB=NB, npc_pad=npc_pad,
               KsL=KsL, KsH=KsH, soL=soL, soH=soH, koL=koL, koH=koH,
               StotL16=StotL16, StotH16=StotH16, KtotL=KtotL, KtotH=KtotH,
               n_chunks=n_chunks, N_pad=N_pad, L_lo=L_lo, LO=LO,
               lo_rows=LO + 1, hi_rows=max(N_pad - LO, 1) + 1,
               bo_eff=bo_eff)

    in_maps = []
    for c in range(C):
        in_maps.append({
            "xT_full": xT_full,
            "xT_L": np.ascontiguousarray(xT_L[c]),
            "xT_H": np.ascontiguousarray(xT_H[c]),
            "idx_L": np.ascontiguousarray(idxL[c]),
            "idx_H": np.ascontiguousarray(idxH[c]),
            "mask_L": np.ascontiguousarray(maskL[c]),
            "mask_H": np.ascontiguousarray(maskH[c]),
            "merge_idx": np.ascontiguousarray(merge_idx[c]),
            "xrwo": np.ascontiguousarray(xrwo_col[c]),
            "wl": wl, "wr": wr, "bl_row": bl_row, "br_row": br_row,
            "att_rep": att_rep, "i_rep": i_rep, "wo_rep": wo_rep,
        })
    out_nodes = np.stack([core_nodes[c][ordH[c]] for c in range(C)])
    return cfg, in_maps, out_nodes


# --------------------------------------------------------------------------
# Device program
# --------------------------------------------------------------------------

def _build(cfg, lrelu_act=True, debug=False):
    import concourse.bass as bass
    import concourse.bacc as bacc
    import concourse.tile as tile
    from concourse import mybir

    f16, f32, i16 = mybir.dt.float16, mybir.dt.float32, mybir.dt.int16
    AT = mybir.ActivationFunctionType
    OP = mybir.AluOpType
    AX = mybir.AxisListType

    NB = cfg["NB"]
    n_chunks, L_lo = cfg["n_chunks"], cfg["L_lo"]
    npc_pad = cfg["npc_pad"]

    nc = bacc.Bacc("TRN2", target_bir_lowering=False, debug=debug,
                   num_devices=cfg["C"], num_swdge_queues=2)

    xT_full = nc.dram_tensor("xT_full", [P, cfg["N_pad"]], f16, kind="ExternalInput")
    xT_L = nc.dram_tensor("xT_L", [P, npc_pad], f16, kind="ExternalInput")
    xT_H = nc.dram_tensor("xT_H", [P, npc_pad], f16, kind="ExternalInput")
    idx_L_d = nc.dram_tensor("idx_L", [P, cfg["StotL16"]], i16, kind="ExternalInput")
    idx_H_d = nc.dram_tensor("idx_H", [P, cfg["StotH16"]], i16, kind="ExternalInput")
    mask_L_d = nc.dram_tensor("mask_L", [P, cfg["KtotL"]], f16, kind="ExternalInput")
    mask_H_d = nc.dram_tensor("mask_H", [P, cfg["KtotH"]], f16, kind="ExternalInput")
    merge_d = nc.dram_tensor("merge_idx", [P, npc_pad // 16], i16, kind="ExternalInput")
    xrwo_d = nc.dram_tensor("xrwo", [P, NB], mybir.dt.float32, kind="ExternalInput")
    wl_d = nc.dram_tensor("wl", [P, P], f16, kind="ExternalInput")
    wr_d = nc.dram_tensor("wr", [P, P], f16, kind="ExternalInput")
    blr_d = nc.dram_tensor("bl_row", [1, P], f16, kind="ExternalInput")
    brr_d = nc.dram_tensor("br_row", [1, P], f16, kind="ExternalInput")
    attr_d = nc.dram_tensor("att_rep", [P, 4, P], f16, kind="ExternalInput")
    irep_d = nc.dram_tensor("i_rep", [P, 4, P], f16, kind="ExternalInput")
    wo_d = nc.dram_tensor("wo_rep", [P, P], f16, kind="ExternalInput")
    out_d = nc.dram_tensor("out", [npc_pad, 1], f32, kind="ExternalOutput")

    table_lo = nc.dram_tensor("table_lo", [cfg["lo_rows"], P], f16)
    table_hi = nc.dram_tensor("table_hi", [cfg["hi_rows"], P], f16)
    partial = nc.dram_tensor("partial", [npc_pad, PARTW], f32)

    def bc(ap, pattern):
        return bass.AP(tensor=ap.tensor, offset=ap.offset,
                       ap=[list(ap.ap[0])] + [list(p) for p in pattern])

    with tile.TileContext(nc) as tc:
        with tc.tile_pool(name="const", bufs=1) as cp:
            wl_sb = cp.tile([P, P], f16, tag="wl")
            wr_sb = cp.tile([P, P], f16, tag="wr")
            blr_sb = cp.tile([1, P], f16, tag="blr")
            brr_sb = cp.tile([1, P], f16, tag="brr")
            att_sb = cp.tile([P, 4, P], f16, tag="attr")
            irep_sb = cp.tile([P, 4, P], f16, tag="irep")
            wo_sb = cp.tile([P, P], f16, tag="wo")
            idxL_sb = cp.tile([P, cfg["StotL16"]], i16, tag="idxL")
            idxH_sb = cp.tile([P, cfg["StotH16"]], i16, tag="idxH")
            maskL_sb = cp.tile([P, cfg["KtotL"]], f16, tag="maskL")
            maskH_sb = cp.tile([P, cfg["KtotH"]], f16, tag="maskH")
            merge_sb = cp.tile([P, npc_pad // 16], i16, tag="mergei")
            xrwo_sb = cp.tile([P, NB], mybir.dt.float32, tag="xrwo")
            xrT_L = cp.tile([P, NB, P], f16, tag="xrTL")
            xrT_H = cp.tile([P, NB, P], f16, tag="xrTH")
            ones1 = cp.tile([1, P], f16, tag="ones1")
            zrow = cp.tile([1, P], f16, tag="zrow")
            bo_sb = cp.tile([P, 1], mybir.dt.float32, tag="bo")
            out_sb = cp.tile([P, NB], mybir.dt.float32, tag="outsb")

            for t, d in ((wl_sb, wl_d), (wr_sb, wr_d), (blr_sb, blr_d),
                         (brr_sb, brr_d), (att_sb, attr_d), (irep_sb, irep_d),
                         (wo_sb, wo_d), (idxL_sb, idx_L_d), (idxH_sb, idx_H_d),
                         (maskL_sb, mask_L_d), (maskH_sb, mask_H_d),
                         (merge_sb, merge_d), (xrwo_sb, xrwo_d)):
                nc.sync.dma_start(out=t, in_=d.ap())
            nc.vector.memset(ones1, 1.0)
            nc.vector.memset(zrow, 0.0)
            nc.vector.memset(bo_sb, cfg["bo_eff"])
            nc.sync.dma_start(out=table_lo.ap()[0:1, :], in_=zrow)
            nc.sync.dma_start(out=table_hi.ap()[0:1, :], in_=zrow)
            if n_chunks <= L_lo:
                nc.sync.dma_start(out=table_hi.ap()[1:2, :], in_=zrow)

            i128 = irep_sb[:, 0, :]

            # ---------------- phase 1a: xrT chunks (both orders) -----------
            with tc.tile_pool(name="p1l", bufs=3) as lp, \
                 tc.tile_pool(name="p1lp", bufs=4, space="PSUM") as lpp:
                for xt_d, xrT in ((xT_L, xrT_L), (xT_H, xrT_H)):
                    for b in range(NB):
                        xtl = lp.tile([P, P], f16, tag="xtl")
                        nc.sync.dma_start(out=xtl,
                                          in_=xt_d.ap()[:, b * P:(b + 1) * P])
                        ps1 = lpp.tile([P, P], mybir.dt.float32, tag="ps1")
                        nc.tensor.matmul(ps1, wr_sb, xtl, start=True, stop=False)
                        nc.tensor.matmul(ps1, brr_sb, ones1, start=False, stop=True)
                        nc.scalar.copy(xrT[:, b, :], ps1)

            # ---------------- phase 1b: xl tables ----------------
            with tc.tile_pool(name="p1x", bufs=3) as xp, \
                 tc.tile_pool(name="p1p", bufs=4, space="PSUM") as pp, \
                 tc.tile_pool(name="p1c", bufs=3) as cvp:
                for ch in range(n_chunks):
                    xt = xp.tile([P, 4, P], f16, tag="xt")
                    nc.sync.dma_start(
                        out=xt, in_=xT_full.ap()[:, ch * CHUNK:(ch + 1) * CHUNK])
                    ps = pp.tile([P, 4, P], mybir.dt.float32, tag="pch")
                    for i in range(4):
                        nc.tensor.matmul(ps[:, i, :], xt[:, i, :], wl_sb,
                                         start=True, stop=False)
                        nc.tensor.matmul(ps[:, i, :], ones1, blr_sb,
                                         start=False, stop=True)
                    cv = cvp.tile([P, 4, P], f16, tag="cv")
                    if ch % 2 == 0:
                        nc.scalar.copy(cv, ps)
                    else:
                        nc.vector.tensor_copy(cv, ps)
                    if ch < L_lo:
                        r0 = ch * CHUNK + 1
                        dst = table_lo.ap()[r0:r0 + CHUNK, :]
                    else:
                        r0 = ch * CHUNK - cfg["LO"] + 1
                        dst = table_hi.ap()[r0:r0 + CHUNK, :]
                    nc.sync.dma_start(
                        out=dst.rearrange("(i n) f -> n i f", n=P), in_=cv)

            # ---------------- phase 2: the two GAT passes ----------------
            def gat_pass(is_lo, gp, zp, sp, part_sb):
                Ks = cfg["KsL"] if is_lo else cfg["KsH"]
                sos = cfg["soL"] if is_lo else cfg["soH"]
                kos = cfg["koL"] if is_lo else cfg["koH"]
                idx_sb = idxL_sb if is_lo else idxH_sb
                mask_sb = maskL_sb if is_lo else maskH_sb
                xrT = xrT_L if is_lo else xrT_H
                table = table_lo if is_lo else table_hi
                tg = "L" if is_lo else "H"
                for b in range(NB):
                    K = Ks[b]
                    nb4 = K // 4
                    so, ko = sos[b], kos[b]
                    g = gp.tile([P, K, P], f16, tag="g" + tg)
                    for j0 in range(0, K, 8):
                        kc = min(8, K - j0)
                        sc = kc * P
                        nc.gpsimd.dma_gather(
                            out_ap=g[:, j0:j0 + kc, :], in_ap=table.ap(),
                            idxs_ap=idx_sb[:, so + j0 * 8:so + j0 * 8 + sc // 16],
                            num_idxs=sc, num_idxs_reg=sc, elem_size=P,
                            queue_num=0 if is_lo else 1)
                    lr = gp.tile([P, K, P], f16, tag="lr" + tg)
                    zc = gp.tile([P, K, P], f16, tag="zc" + tg)
                    for j in range(nb4):
                        zb = zp.tile([P, 4, P], mybir.dt.float32, tag="zb" + tg)
                        nc.tensor.matmul(zb, i128, g[:, 4 * j:4 * j + 4, :],
                                         start=True, stop=False)
                        nc.tensor.matmul(zb, xrT[:, b, :], irep_sb,
                                         start=False, stop=True)
                        lrj = lr[:, 4 * j:4 * j + 4, :]
                        if lrelu_act:
                            nc.scalar.activation(lrj, zb, AT.Prelu,
                                                 alpha=NEG_SLOPE)
                        else:
                            nc.vector.scalar_tensor_tensor(
                                out=lrj, in0=zb, scalar=NEG_SLOPE, in1=zb,
                                op0=OP.mult, op1=OP.max)
                        nc.scalar.copy(zc[:, 4 * j:4 * j + 4, :], zb)

                    lrv = lr.rearrange("p (a b) f -> p a (b f)", b=4)
                    att_b = bc(att_sb, [[0, nb4], [1, 4 * P]])
                    nc.vector.tensor_mul(lrv, lrv, att_b)
                    e_t = sp.tile([P, K], mybir.dt.float32, tag="e")
                    nc.vector.reduce_sum(out=e_t, in_=lr, axis=AX.X)
                    pp_t = sp.tile([P, K], f16, tag="pp")
                    nc.scalar.activation(pp_t, e_t, AT.Exp)
                    pm = sp.tile([P, K], f16, tag="pm")
                    nc.vector.tensor_mul(pm, pp_t, mask_sb[:, ko:ko + K])
                    # p broadcast along features via ACT copy (keeps the
                    # contrib tensor_mul in 2x mode)
                    prep = gp.tile([P, K, P], f16, tag="pr" + tg)
                    nc.scalar.copy(prep, bc(pm, [[1, K], [0, P]]))
                    nc.vector.tensor_mul(zc, zc, prep)
                    agg = sp.tile([P, P], mybir.dt.float32, tag="agg")
                    zcT = bc(zc, [[1, P], [P, K]])
                    nc.vector.reduce_sum(out=agg, in_=zcT, axis=AX.X)
                    den = sp.tile([P, 64], mybir.dt.float32, tag="den")
                    nc.vector.memset(den, 0.0)
                    nc.vector.reduce_sum(out=den[:, 0:1], in_=pm, axis=AX.X)
                    yield b, agg, den, pm

            with tc.tile_pool(name="gat", bufs=2) as gp, \
                 tc.tile_pool(name="zps", bufs=4, space="PSUM") as zp, \
                 tc.tile_pool(name="sm", bufs=3) as sp:
                # ---- pass L: write partials ----
                for b, agg, den, pm in gat_pass(True, gp, zp, sp, None):
                    nc.sync.dma_start(
                        out=partial.ap()[b * P:(b + 1) * P, 0:P], in_=agg)
                    nc.sync.dma_start(
                        out=partial.ap()[b * P:(b + 1) * P, P:P + 64],
                        in_=den)

                # ---- merge gather: partial rows in H order ----
                part_sb = cp.tile([P, NB, PARTW], mybir.dt.float32, tag="part")
                for j0 in range(0, npc_pad, 1024):
                    sc = min(1024, npc_pad - j0)
                    nc.gpsimd.dma_gather(
                        out_ap=part_sb[:, j0 // P:(j0 + sc) // P, :],
                        in_ap=partial.ap(),
                        idxs_ap=merge_sb[:, j0 // 16:(j0 + sc) // 16],
                        num_idxs=sc, num_idxs_reg=sc, elem_size=PARTW,
                        queue_num=0)

                # ---- pass H: merge + finish ----
                for b, agg, den, pm in gat_pass(False, gp, zp, sp, part_sb):
                    nc.vector.tensor_add(agg, agg, part_sb[:, b, 0:P])
                    dent = sp.tile([P, 1], mybir.dt.float32, tag="dent")
                    nc.vector.tensor_add(dent, den[:, 0:1],
                                         part_sb[:, b, P:P + 1])
                    rden = sp.tile([P, 1], mybir.dt.float32, tag="rden")
                    nc.vector.reciprocal(rden, dent)
                    scr = sp.tile([P, P], mybir.dt.float32, tag="scr")
                    aw = sp.tile([P, 1], mybir.dt.float32, tag="aw")
                    nc.vector.scalar_tensor_tensor(
                        out=scr, in0=agg, scalar=1.0, in1=wo_sb,
                        op0=OP.mult, op1=OP.mult, accum_out=aw)
                    lg = sp.tile([P, 1], mybir.dt.float32, tag="lg")
                    nc.vector.scalar_tensor_tensor(
                        out=lg, in0=aw, scalar=rden, in1=xrwo_sb[:, b:b + 1],
                        op0=OP.mult, op1=OP.subtract)
                    nc.scalar.activation(out_sb[:, b:b + 1], lg, AT.Sigmoid,
                                         bias=bo_sb)

            nc.sync.dma_start(
                out=out_d.ap().rearrange("(b n) o -> n (b o)", n=P),
                in_=out_sb)
    nc.compile()
    return nc


# --------------------------------------------------------------------------
# Entry point
# --------------------------------------------------------------------------

def _run(inputs, trace=False, lrelu_act=True):
    from concourse.bass_utils import run_bass_kernel_spmd

    cfg, in_maps, out_nodes = _plan(**inputs)
    nc = _build(cfg, lrelu_act=lrelu_act)
    res = run_bass_kernel_spmd(nc, in_maps, core_ids=list(range(cfg["C"])),
                               trace=trace)

    N = cfg["N"]
    out = np.zeros((N, 1), dtype=np.float32)
    for c in range(cfg["C"]):
        nodes = out_nodes[c]
        ok = nodes >= 0
        out[nodes[ok], 0] = res.results[c]["out"][ok, 0]
    return out, res


def kernel(**inputs):
    return _run(inputs)[0]



# revision 3
# speedup vs baseline: 1.5258x; 1.5258x over previous
"""GATv2 classifier kernel for Trainium2, 8-core SPMD.

Strategy (v2, run-table gather):
  - Nodes dealt round-robin by descending in-degree across 8 cores; edges
    partitioned by destination so segment-softmax/aggregation stay local.
  - Per core, each destination's incoming-edge sources (self-loop first)
    form a CONTIGUOUS RUN in a DRAM table of xl rows, padded per 128-node
    bucket to K_b = nd*k rows/node (k in {4,8,16} adaptive per bucket).
    The table is built on device (xl = x_S @ Wl + bl) from a host-shipped
    per-edge-ordered copy of x; gathers then need only nd descriptors of
    k rows (k*256B) per node instead of one per edge: ~19K descriptors
    per core vs ~134K -- GpSimd desc-gen (7.9ns/desc) was the wall.
  - The table is split into one DRAM tensor per bucket-group, so int16
    gather indices stay in-window and the tile framework pipelines
    group g's gathers against group g+1's table build.
  - Per bucket [128 dst x K slots]: z = g + xr[dst] (DVE broadcast add),
    lr = Prelu(z) (ACT), e = sum_f att*lr (DVE mult+reduce), exact
    segment max via exp(e - emax) (ACT bias), p-weighted aggregation by
    in-place f16 halving tree (contiguous DVE adds).
  - logit = (sum_f agg*wo)/den - xr.wo + (bias.wo + bo); out = sigmoid.
"""

import math
import os
import sys

import numpy as np

if os.path.isdir("/opt/trn_rl_repo") and "/opt/trn_rl_repo" not in sys.path:
    sys.path.insert(0, "/opt/trn_rl_repo")

P = 128
NEG_SLOPE = 0.2
CHUNK = 512           # table rows per phase-1 chunk
ALPHA = 8.0           # gpsimd ns per gather descriptor
BETA = 5.0            # marginal ns per table slot (DVE/ACT/phase-1)
GROUP_BUCKETS = 8     # max buckets per table-group tensor


def _wrap_idx(grid):
    """[nd,128] desc grid -> dma_gather wrapped idx layout [128, nd*128/16]."""
    flat = grid.reshape(-1).astype(np.int16)
    return np.tile(flat.reshape(-1, 16).T, (8, 1))


# --------------------------------------------------------------------------
# Host-side planning
# --------------------------------------------------------------------------

def _plan(x, edge_index, Wl, bl, Wr, br, att, bias, Wo, bo, n_cores=8):
    N, F = x.shape
    assert F == P
    C = n_cores

    src = np.concatenate([np.asarray(edge_index[0], dtype=np.int64),
                          np.arange(N, dtype=np.int64)])
    dst = np.concatenate([np.asarray(edge_index[1], dtype=np.int64),
                          np.arange(N, dtype=np.int64)])
    deg = np.bincount(dst, minlength=N)          # includes self-loop
    is_self = np.zeros(len(src), dtype=np.int8)
    is_self[-N:] = 0
    # order edges by dst, self-loop first within each segment
    notself = np.ones(len(src), dtype=np.int8)
    notself[-N:] = 0
    e_order = np.lexsort((notself, dst))
    src_sorted = src[e_order].astype(np.int32)
    starts = np.concatenate([[0], np.cumsum(deg)]).astype(np.int64)

    # deal nodes round-robin by descending total degree
    order = np.argsort(-deg, kind="stable")
    npc = (N + C - 1) // C
    NB = (npc + P - 1) // P
    npc_pad = NB * P
    order_pad = np.full(C * npc_pad, -1, dtype=np.int64)
    order_pad[:N] = order
    core_nodes = np.stack([order_pad[c::C] for c in range(C)])  # [C, npc_pad]

    # per-core: sort by degree desc -> bucket grid
    nds_all = np.zeros((C, npc_pad), dtype=np.int64)
    degs_all = np.zeros((C, npc_pad), dtype=np.int64)
    for c in range(C):
        nodes = core_nodes[c]
        key = np.where(nodes >= 0, deg[np.maximum(nodes, 0)], -1)
        o = np.argsort(-key, kind="stable")
        nds_all[c] = nodes[o]
        degs_all[c] = np.where(nds_all[c] >= 0,
                               deg[np.maximum(nds_all[c], 0)], 0)

    # global (cross-core) per-bucket schedule: k and nd from max degree
    ks, nd_s = [], []
    for b in range(NB):
        dmax = max(1, int(degs_all[:, b * P:(b + 1) * P].max()))
        best = None
        for k in (4, 8, 16):
            nd = (dmax + k - 1) // k
            cost = nd * (ALPHA + k * BETA)
            if best is None or cost < best[0]:
                best = (cost, k, nd)
        ks.append(best[1])
        nd_s.append(best[2])
    Ks = [ks[b] * nd_s[b] for b in range(NB)]

    # groups of consecutive buckets with equal k
    groups = []       # list of (k, [bucket ids])
    for b in range(NB):
        if groups and groups[-1][0] == ks[b] and len(groups[-1][1]) < GROUP_BUCKETS:
            groups[-1][1].append(b)
        else:
            groups.append((ks[b], [b]))

    # table layout per group
    g_rows = []        # padded rows per group
    g_units = []
    b_rowoff = [0] * NB   # bucket row offset within its group
    b_group = [0] * NB
    for gi, (k, bl_) in enumerate(groups):
        r = 0
        for b in bl_:
            b_group[b] = gi
            b_rowoff[b] = r
            r += P * Ks[b]
        rpad = ((r + CHUNK - 1) // CHUNK) * CHUNK
        g_rows.append(rpad)
        g_units.append(rpad // k)
    Stot = sum(g_rows)
    g_coloff = np.concatenate([[0], np.cumsum(g_rows)]).astype(np.int64)

    # idx / mask / run-source layout (idx shared across cores; masks+runs per core)
    ko = [0] * NB
    io = [0] * NB
    kacc = iacc = 0
    for b in range(NB):
        ko[b] = kacc
        io[b] = iacc
        kacc += Ks[b]
        iacc += (nd_s[b] * P) // 16
    Ktot, I16 = kacc, iacc

    idx_arr = np.zeros((P, I16), dtype=np.int16)
    for b in range(NB):
        k, nd = ks[b], nd_s[b]
        ub = b_rowoff[b] // k
        grid = (ub + np.arange(P)[None, :] * nd + np.arange(nd)[:, None])
        idx_arr[:, io[b]:io[b] + (nd * P) // 16] = _wrap_idx(grid)

    mask = np.zeros((C, P, Ktot), dtype=np.float16)
    src_run = np.full((C, Stot), -1, dtype=np.int64)
    for c in range(C):
        nds = nds_all[c]
        degs = degs_all[c]
        for b in range(NB):
            K = Ks[b]
            base = g_coloff[b_group[b]] + b_rowoff[b]
            blk = nds[b * P:(b + 1) * P]
            db = degs[b * P:(b + 1) * P]
            kk = np.arange(K)
            m = (kk[None, :] < db[:, None]).astype(np.float16)
            m[blk < 0, 0] = 1.0          # dummy slot keeps den > 0
            mask[c, :, ko[b]:ko[b] + K] = m
            # runs
            for p in range(P):
                n = blk[p]
                if n < 0:
                    continue
                d = int(db[p])
                s0 = starts[n]
                src_run[c, base + p * K: base + p * K + d] = \
                    src_sorted[s0:s0 + d]

    x16 = np.asarray(x, dtype=np.float16)
    xT_S = np.zeros((C, P, Stot), dtype=np.float16)
    xdT = np.zeros((C, P, npc_pad), dtype=np.float16)
    for c in range(C):
        okr = src_run[c] >= 0
        xT_S[c][:, okr] = x16[src_run[c][okr]].T
        okn = nds_all[c] >= 0
        xdT[c][:, okn] = x16[nds_all[c][okn]].T

    wl = np.asarray(Wl, dtype=np.float16)
    wr = np.asarray(Wr, dtype=np.float16)
    bl_row = np.asarray(bl, dtype=np.float16).reshape(1, P)
    br_row = np.asarray(br, dtype=np.float16).reshape(1, P)
    att_rep = np.tile(np.asarray(att, dtype=np.float16)[None, :], (P, 1))
    wo_rep = np.tile(np.asarray(Wo, dtype=np.float16)[:, 0][None, :], (P, 1))
    bo_eff = float(np.asarray(bo).reshape(-1)[0] +
                   np.asarray(bias, dtype=np.float64)
                   @ np.asarray(Wo, dtype=np.float64)[:, 0])

    cfg = dict(N=N, C=C, NB=NB, npc_pad=npc_pad, Stot=Stot,
               ks=ks, nd_s=nd_s, Ks=Ks, ko=ko, io=io,
               groups=groups, g_rows=g_rows, g_units=g_units,
               g_coloff=[int(v) for v in g_coloff],
               Ktot=Ktot, I16=I16, bo_eff=bo_eff,
               bl_nz=bool(np.any(np.asarray(bl) != 0)),
               br_nz=bool(np.any(np.asarray(br) != 0)))

    in_maps = []
    for c in range(C):
        in_maps.append({
            "xT_S": np.ascontiguousarray(xT_S[c]),
            "xdT": np.ascontiguousarray(xdT[c]),
            "idx": idx_arr,
            "mask": np.ascontiguousarray(mask[c]),
            "wl": wl, "wr": wr, "bl_row": bl_row, "br_row": br_row,
            "att_rep": att_rep, "wo_rep": wo_rep,
        })
    return cfg, in_maps, nds_all


# --------------------------------------------------------------------------
# Device program
# --------------------------------------------------------------------------

def _build(cfg, lrelu_act=True, debug=False):
    import concourse.bass as bass
    import concourse.bacc as bacc
    import concourse.tile as tile
    from concourse import mybir

    f16, f32, i16 = mybir.dt.float16, mybir.dt.float32, mybir.dt.int16
    AT = mybir.ActivationFunctionType
    OP = mybir.AluOpType
    AX = mybir.AxisListType

    NB = cfg["NB"]
    npc_pad = cfg["npc_pad"]
    ks, nd_s, Ks, ko, io = cfg["ks"], cfg["nd_s"], cfg["Ks"], cfg["ko"], cfg["io"]
    groups, g_rows = cfg["groups"], cfg["g_rows"]
    g_coloff = cfg["g_coloff"]

    nc = bacc.Bacc("TRN2", target_bir_lowering=False, debug=debug,
                   num_devices=cfg["C"], num_swdge_queues=2)

    xT_S_d = nc.dram_tensor("xT_S", [P, cfg["Stot"]], f16, kind="ExternalInput")
    xdT_d = nc.dram_tensor("xdT", [P, npc_pad], f16, kind="ExternalInput")
    idx_d = nc.dram_tensor("idx", [P, cfg["I16"]], i16, kind="ExternalInput")
    mask_d = nc.dram_tensor("mask", [P, cfg["Ktot"]], f16, kind="ExternalInput")
    wl_d = nc.dram_tensor("wl", [P, P], f16, kind="ExternalInput")
    wr_d = nc.dram_tensor("wr", [P, P], f16, kind="ExternalInput")
    blr_d = nc.dram_tensor("bl_row", [1, P], f16, kind="ExternalInput")
    brr_d = nc.dram_tensor("br_row", [1, P], f16, kind="ExternalInput")
    attr_d = nc.dram_tensor("att_rep", [P, P], f16, kind="ExternalInput")
    wo_d = nc.dram_tensor("wo_rep", [P, P], f16, kind="ExternalInput")
    out_d = nc.dram_tensor("out", [npc_pad, 1], f32, kind="ExternalOutput")

    tables = [nc.dram_tensor(f"table{gi}", [g_rows[gi], P], f16)
              for gi in range(len(groups))]

    def bc(ap, pattern):
        return bass.AP(tensor=ap.tensor, offset=ap.offset,
                       ap=[list(ap.ap[0])] + [list(p) for p in pattern])

    with tile.TileContext(nc) as tc:
        with tc.tile_pool(name="const", bufs=1) as cp:
            wl_sb = cp.tile([P, P], f16, tag="wl")
            wr_sb = cp.tile([P, P], f16, tag="wr")
            blr_sb = cp.tile([1, P], f16, tag="blr")
            brr_sb = cp.tile([1, P], f16, tag="brr")
            att_sb = cp.tile([P, P], f16, tag="attr")
            wo_sb = cp.tile([P, P], f16, tag="wo")
            idx_sb = cp.tile([P, cfg["I16"]], i16, tag="idx")
            mask_sb = cp.tile([P, cfg["Ktot"]], f16, tag="mask")
            ones1 = cp.tile([1, P], f16, tag="ones1")
            bo_sb = cp.tile([P, 1], f32, tag="bo")
            out_sb = cp.tile([P, NB], f32, tag="outsb")
            xr_pd = cp.tile([P, NB, P], f16, tag="xrpd")
            xrwo = cp.tile([P, NB], f32, tag="xrwo")

            for t, d in ((wl_sb, wl_d), (wr_sb, wr_d), (blr_sb, blr_d),
                         (brr_sb, brr_d), (att_sb, attr_d), (wo_sb, wo_d),
                         (idx_sb, idx_d), (mask_sb, mask_d)):
                nc.sync.dma_start(out=t, in_=d.ap())
            nc.vector.memset(ones1, 1.0)
            nc.vector.memset(bo_sb, cfg["bo_eff"])

            # ---------------- phase A: xr = x_dst @ Wr + br ----------------
            with tc.tile_pool(name="pa", bufs=3) as ap_, \
                 tc.tile_pool(name="pap", bufs=4, space="PSUM") as app, \
                 tc.tile_pool(name="pas", bufs=3) as asp:
                for b in range(NB):
                    xdt = ap_.tile([P, P], f16, tag="xdt")
                    nc.sync.dma_start(out=xdt,
                                      in_=xdT_d.ap()[:, b * P:(b + 1) * P])
                    ps = app.tile([P, P], f32, tag="psA")
                    if cfg["br_nz"]:
                        nc.tensor.matmul(ps, xdt, wr_sb, start=True, stop=False)
                        nc.tensor.matmul(ps, ones1, brr_sb, start=False, stop=True)
                    else:
                        nc.tensor.matmul(ps, xdt, wr_sb, start=True, stop=True)
                    nc.scalar.copy(xr_pd[:, b, :], ps)
                    scr = asp.tile([P, P], f32, tag="scrA")
                    nc.vector.scalar_tensor_tensor(
                        out=scr, in0=xr_pd[:, b, :], scalar=1.0, in1=wo_sb,
                        op0=OP.mult, op1=OP.mult, accum_out=xrwo[:, b:b + 1])

            # ---------------- phase B: tables + buckets ----------------
            with tc.tile_pool(name="pbx", bufs=3) as xp, \
                 tc.tile_pool(name="pbp", bufs=4, space="PSUM") as pp, \
                 tc.tile_pool(name="pbc", bufs=3) as cvp, \
                 tc.tile_pool(name="gat", bufs=3) as gp, \
                 tc.tile_pool(name="lrp", bufs=3) as lp, \
                 tc.tile_pool(name="sm", bufs=4) as sp:
                cpeng = 0
                for gi, (k, bl_) in enumerate(groups):
                    goff = g_coloff[gi]
                    n_ch = g_rows[gi] // CHUNK
                    for ch in range(n_ch):
                        xt = xp.tile([P, 4, P], f16, tag="xt")
                        nc.sync.dma_start(
                            out=xt,
                            in_=xT_S_d.ap()[:, goff + ch * CHUNK:
                                            goff + (ch + 1) * CHUNK])
                        ps = pp.tile([P, 4, P], f32, tag="psB")
                        for i in range(4):
                            if cfg["bl_nz"]:
                                nc.tensor.matmul(ps[:, i, :], xt[:, i, :],
                                                 wl_sb, start=True, stop=False)
                                nc.tensor.matmul(ps[:, i, :], ones1, blr_sb,
                                                 start=False, stop=True)
                            else:
                                nc.tensor.matmul(ps[:, i, :], xt[:, i, :],
                                                 wl_sb, start=True, stop=True)
                        cv = cvp.tile([P, 4, P], f16, tag="cv")
                        if cpeng == 0:
                            nc.scalar.copy(cv, ps)
                        else:
                            nc.vector.tensor_copy(cv, ps)
                        cpeng = (cpeng + 1) % 2
                        dstap = tables[gi].ap()[ch * CHUNK:(ch + 1) * CHUNK, :]
                        nc.sync.dma_start(
                            out=dstap.rearrange("(i n) f -> n i f", n=P),
                            in_=cv)

                    for b in bl_:
                        nd, K = nd_s[b], Ks[b]
                        g_t = gp.tile([P, K, P], f16, tag="g")
                        tab = tables[gi].ap()
                        in_ap = bass.AP(tensor=tab.tensor, offset=tab.offset,
                                        ap=[[k * P, g_rows[gi] // k],
                                            [1, k * P]])
                        out_ap = bass.AP(tensor=g_t.tensor, offset=g_t.offset,
                                         ap=[list(g_t.ap[0]),
                                             [k * P, nd], [1, k * P]])
                        nc.gpsimd.dma_gather(
                            out_ap=out_ap, in_ap=in_ap,
                            idxs_ap=idx_sb[:, io[b]:io[b] + (nd * P) // 16],
                            num_idxs=nd * P, num_idxs_reg=nd * P,
                            elem_size=k * P, queue_num=b % 2)

                        # z = g + xr[dst]
                        xr_b = bc(xr_pd[:, b, :], [[0, K], [1, P]])
                        nc.vector.tensor_add(g_t, g_t, xr_b)
                        # lr = att * leaky_relu(z)
                        lr = lp.tile([P, K, P], f16, tag="lr")
                        if lrelu_act:
                            nc.scalar.activation(lr, g_t, AT.Prelu,
                                                 alpha=NEG_SLOPE)
                        else:
                            nc.vector.scalar_tensor_tensor(
                                out=lr, in0=g_t, scalar=NEG_SLOPE, in1=g_t,
                                op0=OP.mult, op1=OP.max)
                        att_b = bc(att_sb, [[0, K], [1, P]])
                        nc.vector.tensor_mul(lr, lr, att_b)
                        e_t = sp.tile([P, K], f32, tag="e")
                        nc.vector.reduce_sum(out=e_t, in_=lr, axis=AX.X)
                        nc.vector.tensor_mul(e_t, e_t, mask_sb[:, ko[b]:ko[b] + K])
                        nmax = sp.tile([P, 1], f32, tag="nmax")
                        nc.vector.reduce_max(out=nmax, in_=e_t, axis=AX.X,
                                             negate=True)
                        pp_t = sp.tile([P, K], f16, tag="pp")
                        nc.scalar.activation(pp_t, e_t, AT.Exp, bias=nmax)
                        nc.vector.tensor_mul(pp_t, pp_t,
                                             mask_sb[:, ko[b]:ko[b] + K])
                        den = sp.tile([P, 1], f32, tag="den")
                        nc.vector.reduce_sum(out=den, in_=pp_t, axis=AX.X)
                        rden = sp.tile([P, 1], f32, tag="rden")
                        nc.vector.reciprocal(rden, den)
                        # p-weighted slots, then halving-tree reduce
                        pm_b = bc(pp_t, [[1, K], [0, P]])
                        nc.vector.tensor_mul(g_t, g_t, pm_b)
                        s = K
                        while s > 1:
                            h = (s + 1) // 2
                            w = s - h
                            nc.vector.tensor_add(
                                g_t[:, 0:w, :], g_t[:, 0:w, :], g_t[:, h:s, :])
                            s = h
                        scr = sp.tile([P, P], f32, tag="scr")
                        aw = sp.tile([P, 1], f32, tag="aw")
                        nc.vector.scalar_tensor_tensor(
                            out=scr, in0=g_t[:, 0, :], scalar=1.0, in1=wo_sb,
                            op0=OP.mult, op1=OP.mult, accum_out=aw)
                        lg = sp.tile([P, 1], f32, tag="lg")
                        nc.vector.scalar_tensor_tensor(
                            out=lg, in0=aw, scalar=rden, in1=xrwo[:, b:b + 1],
                            op0=OP.mult, op1=OP.subtract)
                        nc.scalar.activation(out_sb[:, b:b + 1], lg, AT.Sigmoid,
                                             bias=bo_sb)

            nc.sync.dma_start(
                out=out_d.ap().rearrange("(b n) o -> n (b o)", n=P),
                in_=out_sb)
    nc.compile()
    return nc


# --------------------------------------------------------------------------
# Entry point
# --------------------------------------------------------------------------

def _run(inputs, trace=False, lrelu_act=True):
    from concourse.bass_utils import run_bass_kernel_spmd

    cfg, in_maps, out_nodes = _plan(**inputs)
    nc = _build(cfg, lrelu_act=lrelu_act)
    res = run_bass_kernel_spmd(nc, in_maps, core_ids=list(range(cfg["C"])),
                               trace=trace)

    N = cfg["N"]
    out = np.zeros((N, 1), dtype=np.float32)
    for c in range(cfg["C"]):
        nodes = out_nodes[c]
        ok = nodes >= 0
        out[nodes[ok], 0] = res.results[c]["out"][ok, 0]
    return out, res


def kernel(**inputs):
    return _run(inputs)[0]
